# revision 4
# baseline (speedup 1.0000x reference)
"""Causal dot-product attention on 8 Trainium2 NeuronCores.

Problem: q,k,v [16, 2048, 128] fp32, causal softmax(q k^T / sqrt(128)) v.
Sharding: heads (N=16) split across 8 cores, 2 heads per core; no cross-core
communication.

Per-core kernel design (two heads, one per pass, pipelined):
  - Q and K are transposed to [F, T] float32r layout via chunked PE transposes
    driven by a float32r identity (1.5 cycles/row vs fp32's 2). V is cast to
    bf16 with an all-ones column appended, so the attention matmul itself
    produces the softmax row-sums.
  - Scores are computed transposed, scoresT[s, q] = kT_j.T @ qT, in j-groups
    that alternate between a 4-bank "quad" PSUM buffer (4 off-diagonal j-tiles
    exp'd by ONE wide activation) and a 2-bank "pair" buffer (diagonal
    j-tiles, width-trimmed to the causal span). The scalar engine's exp has a
    ~352-cycle per-instruction overhead, so fewer/wider activations matter.
  - QK matmuls are width-trimmed per-j on diagonal tiles (columns below the
    causal span are never computed; exp may read stale-but-finite PSUM there,
    and those expT lanes are never consumed by AV).
  - The causal band of diagonal tiles is zeroed post-exp by gpsimd
    affine_select (index-based, data-independent).
  - out[q, f+1] accumulates expT_ij.T @ [v_j | 1] over j into 2 PSUM banks
    (no start=True: a start clears the whole bank's has_written bits, so the
    banks are pre-zeroed and every matmul accumulates). Column 128 is the
    softmax denominator; normalize = per-partition reciprocal + scalar-mul,
    deferred off the block-boundary critical path.
  - AV matmuls for group g are deferred behind the QK+exp of the next
    AV_DEPTH groups so the in-order PE queue always has ready QK work while
    exp runs; chunk loads/transposes for the next block (or next head) are
    injected mid-block so DMA + PE-transpose + DVE-copy overlap the matmuls.
"""

import numpy as np

import concourse.bass as bass
import concourse.mybir as mybir
import concourse.tile as tile
from concourse import bacc
from concourse.bass import ts
from concourse.bass_utils import run_bass_kernel_spmd
from concourse.masks import make_identity
from concourse.tile_rust import add_dep_helper

N, T, F = 16, 2048, 128
N_CORES = 8
H = N // N_CORES  # heads per core
P = 128
NT = T // P  # 16 k/q tiles per head
BLK = 4  # q-tiles per block (512 q columns)
NBLK = NT // BLK
SCALE = 1.0 / float(np.sqrt(F))
F32 = mybir.dt.float32
F32R = mybir.dt.float32r  # TF32-like PE mode: 1 cycle/row at N>=256 (fp32 is 4)
BF16 = mybir.dt.bfloat16

AV_DEPTH = 2  # deferred-AV depth in groups (2 rotating score buffers)


def build(masked: bool):
    nc = bacc.Bacc("TRN2", target_bir_lowering=False, debug=False, num_devices=N_CORES)
    q = nc.dram_tensor("q", [H, T, F], F32, kind="ExternalInput")
    k = nc.dram_tensor("k", [H, T, F], F32, kind="ExternalInput")
    v = nc.dram_tensor("v", [H, T, F], F32, kind="ExternalInput")
    out = nc.dram_tensor("out", [H, T, F], F32, kind="ExternalOutput")

    with tile.TileContext(nc) as tc:
        _attention(tc, out, q, k, v, masked)
    nc.compile()
    return nc


def plan_groups(b: int, masked: bool):
    """j-tile groups for block b, alternating the quad (Q) and pair (P) PSUM
    buffers. Each group: (buf, js, qk_lo per j, exp_lo)."""
    if not masked:
        return [
            ("Q", [0, 1, 2, 3], [0, 0, 0, 0], 0),
            ("P", [4, 5], [0, 0], 0),
            ("Q", [6, 7, 8, 9], [0, 0, 0, 0], 0),
            ("P", [10, 11], [0, 0], 0),
            ("Q", [12, 13, 14, 15], [0, 0, 0, 0], 0),
        ]
    d = 4 * b
    d1 = ([d + 0, d + 1], [0, 128], 0)
    d2 = ([d + 2, d + 3], [256, 384], 256)
    off = [list(range(4 * c, 4 * c + 4)) for c in range(b)]
    if b == 0:
        return [("Q",) + d1, ("P",) + d2]
    if b == 1:
        return [("Q", off[0], [0] * 4, 0), ("P",) + d1, ("Q",) + d2]
    if b == 2:
        return [
            ("Q", off[0], [0] * 4, 0), ("P",) + d1,
            ("Q", off[1], [0] * 4, 0), ("P",) + d2,
        ]
    return [
        ("Q", off[0], [0] * 4, 0), ("P",) + d1,
        ("Q", off[1], [0] * 4, 0), ("P",) + d2,
        ("Q", off[2], [0] * 4, 0),
    ]


def _attention(tc, out, q, k, v, masked: bool):
    from contextlib import ExitStack

    nc = tc.nc
    ctx = ExitStack()
    consts = ctx.enter_context(tc.tile_pool(name="consts", bufs=1))
    nat_pool = ctx.enter_context(tc.tile_pool(name="nat", bufs=4))
    big_pool = ctx.enter_context(tc.tile_pool(name="big", bufs=2))
    vpool = ctx.enter_context(tc.tile_pool(name="vpool", bufs=2))
    exp_pool = ctx.enter_context(tc.tile_pool(name="expp", bufs=5))
    osb_pool = ctx.enter_context(tc.tile_pool(name="osb", bufs=2))
    rec_pool = ctx.enter_context(tc.tile_pool(name="rec", bufs=4))
    ps_q = ctx.enter_context(tc.tile_pool(name="ps_q", bufs=1, space="PSUM"))
    ps_p = ctx.enter_context(tc.tile_pool(name="ps_p", bufs=1, space="PSUM"))
    ps_acc = ctx.enter_context(tc.tile_pool(name="ps_acc", bufs=1, space="PSUM"))

    identity = consts.tile([P, P], F32)
    make_identity(nc, identity[:])
    # touch Exp once at t=0 so the ~2.7us ACT table load overlaps the first
    # input DMA instead of delaying the first real exp
    warm = consts.tile([P, 1], F32)
    nc.scalar.activation(warm[:], identity[:, 0:1], mybir.ActivationFunctionType.Exp)
    # warm the PE HAM clock gate during the initial input-DMA wait: ~2us of
    # dummy transposes push the activity window over its busy threshold so
    # the first real transposes/matmuls run at 2.4 GHz instead of 1.2
    wtp = ps_q.tile([P, P], F32, tag="Q", name="wtp")
    for _ in range(6):
        nc.tensor.transpose(wtp[:], identity[:], identity[:])

    q_ap, k_ap, v_ap, out_ap = q[:], k[:], v[:], out[:]
    CH = 4  # tiles per dma/transpose chunk (= one q-block's worth)

    def load_transpose_chunk(r3, dst, c, eng=None):
        """DMA 4 natural [128,128] tiles and PE-transpose them into dst.

        eng picks the issuing HWDGE ring — HWDGE DMAs are FIFO per issuing
        engine, so the cold-start K and Q chunks go on different rings
        (sync vs scalar) to transfer in parallel.
        """
        nat = nat_pool.tile([P, CH, P], F32, tag="nat")
        (eng or nc.sync).dma_start(
            out=nat[:], in_=r3[:, c * CH : (c + 1) * CH, :]
        )
        tp = ps_p.tile([P, CH, P], F32, tag="P", name="tp")
        for u in range(CH):
            nc.tensor.transpose(tp[:, u, :], nat[:, u, :], identity[:])
        nc.vector.tensor_copy(dst[:, c * CH * P : (c + 1) * CH * P], tp[:])

    def mk_state(n):
        st = {
            "n": n,
            "kr3": k_ap[n].rearrange("(j p) f -> p j f", p=P),
            "qr3": q_ap[n].rearrange("(j p) f -> p j f", p=P),
            "vr3": v_ap[n].rearrange("(j p) f -> p j f", p=P),
            "kT": big_pool.tile([P, T], F32R, tag="kT", name="kT"),
            "qT": big_pool.tile([P, T], F32R, tag="qT", name="qT"),
            "v_aug": vpool.tile([P, NT, P + 1], BF16, tag="vaug", name="v_aug"),
            "out_sb": osb_pool.tile([P, NT, P], F32, tag="osb", name="out_sb"),
        }
        nc.vector.memset(st["v_aug"][:, :, P : P + 1], 1.0)
        return st

    def load_chunks(st, c, kv=True, cold=False):
        if kv:
            load_transpose_chunk(st["kr3"], st["kT"], c)
            # SWDGE casts fp32 -> bf16 in flight
            nc.gpsimd.dma_start(
                out=st["v_aug"][:, c * CH : (c + 1) * CH, 0:P],
                in_=st["vr3"][:, c * CH : (c + 1) * CH, :],
            )
        load_transpose_chunk(
            st["qr3"], st["qT"], c, eng=nc.scalar if cold else None
        )

    def normalize_and_store(st, acc_sb, b):
        rec4 = rec_pool.tile([P, BLK], F32, tag="rec")
        nc.vector.reciprocal(rec4[:], acc_sb[:, :, P : P + 1])
        for ii in range(BLK):
            i = BLK * b + ii
            nc.vector.tensor_scalar_mul(
                st["out_sb"][:, i, :], acc_sb[:, ii, 0:P], rec4[:, ii : ii + 1]
            )
        nc.sync.dma_start(
            out=out_ap[st["n"]].rearrange("(i p) f -> p i f", p=P)[
                :, BLK * b : BLK * (b + 1), :
            ],
            in_=st["out_sb"][:, BLK * b : BLK * (b + 1), :],
        )

    pending = []
    deferred = []

    def flush_one():
        av_fn, last_of_block, accs_, st_, b_ = deferred.pop(0)
        av_fn()
        if last_of_block:
            # evacuate accumulators; normalize is deferred further still
            acc_sb = rec_pool.tile([P, BLK, P + 1], F32, tag="accsb", name="acc_sb")
            nc.vector.tensor_copy(acc_sb[:], accs_[:, :, 0 : P + 1])
            pending.append((st_, acc_sb, b_))

    def flush_av():
        while deferred:
            flush_one()

    st = None
    st_next = None
    for n in range(H):
        st, st_next = st_next, None
        if st is None:
            st = mk_state(n)
            load_chunks(st, 0, cold=True)
        if not masked:
            for c in range(1, NBLK):
                load_transpose_chunk(st["kr3"], st["kT"], c)
                nc.gpsimd.dma_start(
                    out=st["v_aug"][:, c * CH : (c + 1) * CH, 0:P],
                    in_=st["vr3"][:, c * CH : (c + 1) * CH, :],
                )
        for b in range(NBLK):
            groups = plan_groups(b, masked)
            # last (group, j) hitting each accumulator, for stop flags
            last_map = {}
            for gi, (_, js, _, _) in enumerate(groups):
                for j in js:
                    for ii in range(BLK):
                        if not masked or j <= BLK * b + ii:
                            last_map[ii] = (gi, j)
            # Accumulators all share 2 PSUM banks at 256-fp32 stride.
            # start=True clears the whole bank's has_written bits, so only
            # the first j=0 matmul of each BANK starts (clearing the bank);
            # the neighbour accumulator's j=0 matmul is explicitly ordered
            # after it and overwrites (its hw bit was just cleared).
            accs = ps_acc.tile([P, BLK, 256], F32, tag="acc")  # 2 PSUM banks
            bank_first = {}
            inject_at = min(1, len(groups) - 1)
            for gi, (buf, js, qk_lo, exp_lo) in enumerate(groups):
                while len(deferred) >= AV_DEPTH:
                    flush_one()
                g = len(js)
                pool = ps_q if buf == "Q" else ps_p
                scores = pool.tile([P, g, 512], F32, tag=buf)
                for r, j in enumerate(js):
                    lo = qk_lo[r]
                    nc.tensor.matmul(
                        scores[:, r, lo:512],
                        lhsT=st["kT"][:, ts(j, P)],
                        rhs=st["qT"][:, 512 * b + lo : 512 * (b + 1)],
                        start=True,
                        stop=True,
                    )
                expT = exp_pool.tile([P, g, 512], BF16, tag="e")
                nc.scalar.activation(
                    expT[:, 0:g, exp_lo:512],
                    scores[:, 0:g, exp_lo:512],
                    mybir.ActivationFunctionType.Exp,
                    scale=SCALE,
                )
                if masked:
                    # zero the upper-triangular (non-causal) band of any
                    # diagonal tile, post-exp, on the otherwise-idle gpsimd
                    for r, j in enumerate(js):
                        ii = j - BLK * b
                        if 0 <= ii < BLK:
                            nc.gpsimd.affine_select(
                                out=expT[:, r, ts(ii, P)],
                                in_=expT[:, r, ts(ii, P)],
                                compare_op=mybir.AluOpType.is_ge,
                                fill=0.0,
                                base=0,
                                pattern=[[1, P]],
                                channel_multiplier=-1,
                            )

                def av_fn(expT=expT, js=js, gi=gi, accs=accs, st=st, b=b,
                          bank_first=bank_first, last_map=last_map):
                    for r, j in enumerate(js):
                        for ii in range(BLK):
                            if masked and j > BLK * b + ii:
                                continue
                            bank = ii // 2
                            first = j == 0 and bank not in bank_first
                            m = nc.tensor.matmul(
                                accs[:, ii, 0 : P + 1],
                                lhsT=expT[:, r, ts(ii, P)],
                                rhs=st["v_aug"][:, j, :],
                                start=first,
                                stop=last_map[ii] == (gi, j),
                                skip_group_check=True,
                            )
                            if first:
                                bank_first[bank] = m
                            elif j == 0:
                                # the bank-clearing start above must execute
                                # before this overwrite of the cleared bank
                                add_dep_helper(
                                    m.ins,
                                    bank_first[bank].ins,
                                    reason="acc bank clear precedes neighbour j0",
                                )

                deferred.append((av_fn, gi == len(groups) - 1, accs, st, b))
                if gi == inject_at:
                    # mid-block: previous block's normalize + next block's
                    # (or next head's) chunk loads run here, clear of the
                    # boundary handoff
                    while pending:
                        normalize_and_store(*pending.pop(0))
                    if b + 1 < NBLK:
                        load_chunks(st, b + 1, kv=masked)
                    elif n + 1 < H:
                        st_next = mk_state(n + 1)
                        load_chunks(st_next, 0)
    flush_av()
    while pending:
        normalize_and_store(*pending.pop(0))

    ctx.close()


_CACHE = {}


def _get_nc(masked: bool):
    key = bool(masked)
    if key not in _CACHE:
        _CACHE[key] = build(key)
    return _CACHE[key]


def _run(q, k, v, masked, **kwargs):
    nc = _get_nc(masked)
    q = np.ascontiguousarray(np.asarray(q, dtype=np.float32))
    k = np.ascontiguousarray(np.asarray(k, dtype=np.float32))
    v = np.ascontiguousarray(np.asarray(v, dtype=np.float32))
    in_maps = [
        {
            "q": q[c * H : (c + 1) * H],
            "k": k[c * H : (c + 1) * H],
            "v": v[c * H : (c + 1) * H],
        }
        for c in range(N_CORES)
    ]
    res = run_bass_kernel_spmd(nc, in_maps, core_ids=list(range(N_CORES)), **kwargs)
    outs = np.concatenate([r["out"] for r in res.results], axis=0)
    return outs, res


def kernel(q, k, v, masked):
    m = int(np.asarray(masked))
    outs, _ = _run(q, k, v, m != 0)
    return outs


if __name__ == "__main__":
    rng = np.random.default_rng(0)
    qq = rng.standard_normal((N, T, F), dtype=np.float32)
    kk = rng.standard_normal((N, T, F), dtype=np.float32)
    vv = rng.standard_normal((N, T, F), dtype=np.float32)
    o = kernel(qq, kk, vv, 1)
    print("out", o.shape, o.dtype, float(np.abs(o).mean()))


# revision 10
# speedup vs baseline: 1.1280x; 1.1280x over previous
"""Causal dot-product attention on 8 Trainium2 NeuronCores.

Problem: q,k,v [16, 2048, 128] fp32, causal softmax(q k^T / sqrt(128)) v.
Sharding: heads (N=16) split across 8 cores, 2 heads per core; no cross-core
communication.

Per-core kernel design (two heads, one per pass, pipelined):
  - Q and K are DMA'd into float32r-typed tiles (raw-bit view of the fp32
    input) and PE-transposed to [F, T] with a float32r identity: f32r
    transposes are single-pass (fp32 transposes run LOW+HIGH two-pass, 2x
    the cycles). V is cast to bf16 with an all-ones column appended, so the
    attention matmul itself produces the softmax row-sums.
  - Scores are computed transposed, scoresT[s, q] = kT_j.T @ qT, in j-groups
    that alternate between a 4-bank "quad" PSUM buffer (4 off-diagonal
    j-tiles exp'd by ONE wide activation) and a 2-bank "pair" buffer
    (diagonal j-tiles, width-trimmed per-j to the causal span). The scalar
    engine's ACTIVATE has a ~352-cycle fixed overhead, so fewer/wider exps
    matter; PSUM (8 banks: 4 quad + 2 pair + 2 accumulators) bounds width.
  - The causal band of diagonal tiles is zeroed post-exp by gpsimd
    affine_select (index-based, data-independent; stale lanes left by the
    trimmed QK are never read downstream).
  - out[q, f+1] accumulates expT_ij.T @ [v_j | 1] over j into 2 PSUM banks
    (no start=True: a start clears the whole bank's has_written bits, so the
    banks are pre-zeroed and every matmul accumulates). Column 128 is the
    softmax denominator; normalize = per-partition reciprocal + scalar-mul,
    deferred off the block-boundary critical path.
  - AV matmuls for group g are deferred behind later groups' QK+exp so the
    in-order PE queue always has ready work while exp runs; all deferred AV
    is flushed at each block's first group, right before the next chunk's
    DMA + transposes, so the transposes (which stage through the pair PSUM
    slot) never stall the PE: the pair slot was last exp'd a full block ago.
  - Cold start: the first K/Q/V chunk DMAs are issued before any warmup so
    the transfers overlap the exp-table load and the PE clock-ramp dummies.
"""

import numpy as np

import concourse.bass as bass
import concourse.mybir as mybir
import concourse.tile as tile
from concourse import bacc
from concourse.bass import ts
from concourse.bass_utils import run_bass_kernel_spmd
from concourse.masks import make_identity
from concourse.tile_rust import add_dep_helper

N, T, F = 16, 2048, 128
N_CORES = 8
H = N // N_CORES  # heads per core
P = 128
NT = T // P  # 16 k/q tiles per head
BLK = 4  # q-tiles per block (512 q columns)
NBLK = NT // BLK
SCALE = 1.0 / float(np.sqrt(F))
F32 = mybir.dt.float32
F32R = mybir.dt.float32r  # TF32-like PE mode: 1 cycle/row at N>=256 (fp32 is 4)
BF16 = mybir.dt.bfloat16

AV_DEPTH = 2  # deferred-AV depth in groups (2 rotating score buffers)


def build(masked: bool):
    nc = bacc.Bacc("TRN2", target_bir_lowering=False, debug=False, num_devices=N_CORES)
    q = nc.dram_tensor("q", [H, T, F], F32, kind="ExternalInput")
    k = nc.dram_tensor("k", [H, T, F], F32, kind="ExternalInput")
    v = nc.dram_tensor("v", [H, T, F], F32, kind="ExternalInput")
    out = nc.dram_tensor("out", [H, T, F], F32, kind="ExternalOutput")

    with tile.TileContext(nc) as tc:
        _attention(tc, out, q, k, v, masked)
    nc.compile()
    return nc


def plan_groups(b: int, masked: bool):
    """j-tile groups for block b, alternating the quad (Q) and pair (P) PSUM
    buffers. Each group: (buf, js, qk_lo per j, exp_lo). Every block has an
    even group count ending on P, so the alternation is seamless across
    blocks and the Q slot is always two exps old when the next block's first
    QK wants it (no boundary stall). Quads only ever sit on the Q slot."""
    if not masked:
        return [
            ("Q", [0, 1, 2, 3], [0] * 4, 0),
            ("P", [4, 5], [0, 0], 0),
            ("Q", [6, 7, 8, 9], [0] * 4, 0),
            ("P", [10, 11], [0, 0], 0),
            ("Q", [12, 13], [0, 0], 0),
            ("P", [14, 15], [0, 0], 0),
        ]
    d = 4 * b
    d1 = ([d + 0, d + 1], [0, 128], 0)
    d2 = ([d + 2, d + 3], [256, 384], 256)
    q4 = [(list(range(4 * c, 4 * c + 4)), [0] * 4, 0) for c in range(b)]
    if b == 0:
        return [("Q",) + d1, ("P",) + d2]
    if b == 1:
        return [
            ("Q", [0, 1], [0, 0], 0), ("P", [2, 3], [0, 0], 0),
            ("Q",) + d1, ("P",) + d2,
        ]
    if b == 2:
        return [("Q",) + q4[0], ("P",) + d1, ("Q",) + q4[1], ("P",) + d2]
    return [
        ("Q",) + q4[0], ("P", [8, 9], [0, 0], 0),
        ("Q",) + q4[1], ("P", [10, 11], [0, 0], 0),
        ("Q",) + d1, ("P",) + d2,
    ]


def _attention(tc, out, q, k, v, masked: bool):
    from contextlib import ExitStack

    nc = tc.nc
    ctx = ExitStack()
    consts = ctx.enter_context(tc.tile_pool(name="consts", bufs=1))
    nat_pool = ctx.enter_context(tc.tile_pool(name="nat", bufs=4))
    big_pool = ctx.enter_context(tc.tile_pool(name="big", bufs=2))
    vpool = ctx.enter_context(tc.tile_pool(name="vpool", bufs=2))
    exp_pool = ctx.enter_context(tc.tile_pool(name="expp", bufs=5))
    osb_pool = ctx.enter_context(tc.tile_pool(name="osb", bufs=2))
    rec_pool = ctx.enter_context(tc.tile_pool(name="rec", bufs=4))
    ps_q = ctx.enter_context(tc.tile_pool(name="ps_q", bufs=1, space="PSUM"))
    ps_p = ctx.enter_context(tc.tile_pool(name="ps_p", bufs=1, space="PSUM"))
    ps_acc = ctx.enter_context(tc.tile_pool(name="ps_acc", bufs=1, space="PSUM"))

    q_ap, k_ap, v_ap, out_ap = q[:], k[:], v[:], out[:]
    CH = 4  # tiles per dma/transpose chunk (= one q-block's worth)

    def mk_state(n):
        st = {
            "n": n,
            "kr3": k_ap[n].rearrange("(j p) f -> p j f", p=P),
            "qr3": q_ap[n].rearrange("(j p) f -> p j f", p=P),
            "vr3": v_ap[n].rearrange("(j p) f -> p j f", p=P),
            "kT": big_pool.tile([P, T], F32R, tag="kT", name="kT"),
            "qT": big_pool.tile([P, T], F32R, tag="qT", name="qT"),
            "v_aug": vpool.tile([P, NT, P + 1], BF16, tag="vaug", name="v_aug"),
            "out_sb": osb_pool.tile([P, NT, P], F32, tag="osb", name="out_sb"),
        }
        nc.vector.memset(st["v_aug"][:, :, P : P + 1], 1.0)
        return st

    def dma_chunk(r3, c, eng=None):
        """DMA 4 natural [128,128] tiles; the f32r tile is a raw-bit view of
        the fp32 input so the PE transpose runs in single-pass f32r mode."""
        nat = nat_pool.tile([P, CH, P], F32R, tag="nat")
        (eng or nc.sync).dma_start(
            out=nat[:], in_=r3[:, c * CH : (c + 1) * CH, :].bitcast(F32R)
        )
        return nat

    def transpose_chunk(nat, dst, c):
        tp = ps_p.tile([P, CH, P], F32R, tag="P", name="tp")
        for u in range(CH):
            nc.tensor.transpose(tp[:, u, :], nat[:, u, :], ident_r)
        nc.vector.tensor_copy(dst[:, c * CH * P : (c + 1) * CH * P], tp[:])

    def dma_v_chunk(st, c):
        # SWDGE casts fp32 -> bf16 in flight
        nc.gpsimd.dma_start(
            out=st["v_aug"][:, c * CH : (c + 1) * CH, 0:P],
            in_=st["vr3"][:, c * CH : (c + 1) * CH, :],
        )

    def load_chunks(st, c, kv=True):
        if kv:
            kn = dma_chunk(st["kr3"], c)
            dma_v_chunk(st, c)
        qn = dma_chunk(st["qr3"], c)
        if kv:
            transpose_chunk(kn, st["kT"], c)
        transpose_chunk(qn, st["qT"], c)

    def normalize_and_store(st, acc_sb, b):
        rec4 = rec_pool.tile([P, BLK], F32, tag="rec")
        nc.vector.reciprocal(rec4[:], acc_sb[:, :, P : P + 1])
        for ii in range(BLK):
            i = BLK * b + ii
            nc.vector.tensor_scalar_mul(
                st["out_sb"][:, i, :], acc_sb[:, ii, 0:P], rec4[:, ii : ii + 1]
            )
        nc.sync.dma_start(
            out=out_ap[st["n"]].rearrange("(i p) f -> p i f", p=P)[
                :, BLK * b : BLK * (b + 1), :
            ],
            in_=st["out_sb"][:, BLK * b : BLK * (b + 1), :],
        )

    # ---- cold start: first chunk DMAs in flight before any warmup ----
    st = mk_state(0)
    k0 = dma_chunk(st["kr3"], 0)
    q0 = dma_chunk(st["qr3"], 0, eng=nc.scalar)
    dma_v_chunk(st, 0)

    identity_f = consts.tile([P, P], F32)
    make_identity(nc, identity_f[:])
    identity = consts.tile([P, P], F32R)
    nc.vector.tensor_copy(identity[:], identity_f[:])  # fp32 -> f32r rounding
    ident_r = identity[:]
    # touch Exp once at t=0 so the ~2.7us ACT table load overlaps the first
    # input DMA instead of delaying the first real exp
    warm_in = consts.tile([P, 1], F32)
    nc.vector.memset(warm_in[:], 0.0)
    warm = consts.tile([P, 1], F32)
    nc.scalar.activation(warm[:], warm_in[:], mybir.ActivationFunctionType.Exp)
    # warm the PE HAM clock gate during the initial input-DMA wait: ~2us of
    # dummy transposes push the activity window over its busy threshold so
    # the first real transposes/matmuls run at 2.4 GHz instead of 1.2
    wtp = ps_q.tile([P, P], F32R, tag="Q", name="wtp")
    for _ in range(10):
        nc.tensor.transpose(wtp[:], ident_r, ident_r)
    transpose_chunk(k0, st["kT"], 0)
    transpose_chunk(q0, st["qT"], 0)

    pending = []
    deferred = []

    def flush_one():
        av_fn, last_of_block, accs_, st_, b_ = deferred.pop(0)
        av_fn()
        if last_of_block:
            # evacuate accumulators; normalize is deferred further still
            acc_sb = rec_pool.tile([P, BLK, P + 1], F32, tag="accsb", name="acc_sb")
            nc.vector.tensor_copy(acc_sb[:], accs_[:, :, 0 : P + 1])
            pending.append((st_, acc_sb, b_))

    def flush_av():
        while deferred:
            flush_one()

    st_next = None
    for n in range(H):
        if st is None:
            st, st_next = st_next, None
        if not masked:
            for c in range(1, NBLK):
                kn = dma_chunk(st["kr3"], c)
                dma_v_chunk(st, c)
                transpose_chunk(kn, st["kT"], c)
        for b in range(NBLK):
            groups = plan_groups(b, masked)
            # last (group, j) hitting each accumulator, for stop flags
            last_map = {}
            for gi, (_, js, _, _) in enumerate(groups):
                for j in js:
                    for ii in range(BLK):
                        if not masked or j <= BLK * b + ii:
                            last_map[ii] = (gi, j)
            # Accumulators all share 2 PSUM banks at 256-fp32 stride.
            # start=True clears the whole bank's has_written bits, so only
            # the first j=0 matmul of each BANK starts (clearing the bank);
            # the neighbour accumulator's j=0 matmul is explicitly ordered
            # after it and overwrites (its hw bit was just cleared).
            accs = ps_acc.tile([P, BLK, 256], F32, tag="acc")  # 2 PSUM banks
            bank_first = {}
            for gi, (buf, js, qk_lo, exp_lo) in enumerate(groups):
                if gi != 0:
                    while len(deferred) >= AV_DEPTH:
                        flush_one()
                g = len(js)
                pool = ps_q if buf == "Q" else ps_p
                scores = pool.tile([P, g, 512], F32, tag=buf)
                for r, j in enumerate(js):
                    lo = qk_lo[r]
                    nc.tensor.matmul(
                        scores[:, r, lo:512],
                        lhsT=st["kT"][:, ts(j, P)],
                        rhs=st["qT"][:, 512 * b + lo : 512 * (b + 1)],
                        start=True,
                        stop=True,
                    )
                expT = exp_pool.tile([P, g, 512], BF16, tag="e")
                nc.scalar.activation(
                    expT[:, 0:g, exp_lo:512],
                    scores[:, 0:g, exp_lo:512],
                    mybir.ActivationFunctionType.Exp,
                    scale=SCALE,
                )
                if masked:
                    # zero the upper-triangular (non-causal) band of any
                    # diagonal tile, post-exp, on the otherwise-idle gpsimd
                    for r, j in enumerate(js):
                        ii = j - BLK * b
                        if 0 <= ii < BLK:
                            nc.gpsimd.affine_select(
                                out=expT[:, r, ts(ii, P)],
                                in_=expT[:, r, ts(ii, P)],
                                compare_op=mybir.AluOpType.is_ge,
                                fill=0.0,
                                base=0,
                                pattern=[[1, P]],
                                channel_multiplier=-1,
                            )

                def av_fn(expT=expT, js=js, gi=gi, accs=accs, st=st, b=b,
                          bank_first=bank_first, last_map=last_map):
                    for r, j in enumerate(js):
                        for ii in range(BLK):
                            if masked and j > BLK * b + ii:
                                continue
                            bank = ii // 2
                            first = j == 0 and bank not in bank_first
                            m = nc.tensor.matmul(
                                accs[:, ii, 0 : P + 1],
                                lhsT=expT[:, r, ts(ii, P)],
                                rhs=st["v_aug"][:, j, :],
                                start=first,
                                stop=last_map[ii] == (gi, j),
                                skip_group_check=True,
                            )
                            if first:
                                bank_first[bank] = m
                            elif j == 0:
                                # the bank-clearing start above must execute
                                # before this overwrite of the cleared bank
                                add_dep_helper(
                                    m.ins,
                                    bank_first[bank].ins,
                                    reason="acc bank clear precedes neighbour j0",
                                )

                if gi == 0:
                    # the previous block's deferred AV flushes here: ready PE
                    # meat while this group's exp runs. The last deferred
                    # group's exp may still be in flight, so it flushes after
                    # the chunk loads (whose transposes stage through the P
                    # slot, last exp'd two groups ago - no stall either way).
                    while len(deferred) > 1:
                        flush_one()
                    if b + 1 < NBLK:
                        load_chunks(st, b + 1, kv=masked)
                    elif n + 1 < H:
                        st_next = mk_state(n + 1)
                        load_chunks(st_next, 0)
                    flush_av()
                    while pending:
                        normalize_and_store(*pending.pop(0))
                deferred.append((av_fn, gi == len(groups) - 1, accs, st, b))
        st = None
    flush_av()
    while pending:
        normalize_and_store(*pending.pop(0))

    ctx.close()


_CACHE = {}


def _get_nc(masked: bool):
    key = bool(masked)
    if key not in _CACHE:
        _CACHE[key] = build(key)
    return _CACHE[key]


def _run(q, k, v, masked, **kwargs):
    nc = _get_nc(masked)
    q = np.ascontiguousarray(np.asarray(q, dtype=np.float32))
    k = np.ascontiguousarray(np.asarray(k, dtype=np.float32))
    v = np.ascontiguousarray(np.asarray(v, dtype=np.float32))
    in_maps = [
        {
            "q": q[c * H : (c + 1) * H],
            "k": k[c * H : (c + 1) * H],
            "v": v[c * H : (c + 1) * H],
        }
        for c in range(N_CORES)
    ]
    res = run_bass_kernel_spmd(nc, in_maps, core_ids=list(range(N_CORES)), **kwargs)
    outs = np.concatenate([r["out"] for r in res.results], axis=0)
    return outs, res


def kernel(q, k, v, masked):
    m = int(np.asarray(masked))
    outs, _ = _run(q, k, v, m != 0)
    return outs


if __name__ == "__main__":
    rng = np.random.default_rng(0)
    qq = rng.standard_normal((N, T, F), dtype=np.float32)
    kk = rng.standard_normal((N, T, F), dtype=np.float32)
    vv = rng.standard_normal((N, T, F), dtype=np.float32)
    o = kernel(qq, kk, vv, 1)
    print("out", o.shape, o.dtype, float(np.abs(o).mean()))
